# revision 17
# baseline (speedup 1.0000x reference)
"""MoE layer (8 experts, top-2 hash routing) on 8 Trainium2 NeuronCores.

Strategy: shard the FFN along the dff axis (4096 -> 8 slices of 512).
Every core computes, for all routed (token, expert) pairs, the partial
FFN contribution of its dff slice:

    z_core[t] = sum_{e in sel(t)} relu(x[t] @ W1[e][:, S] + b1[e][S]) @ W2[e][S, :]

The host sorts tokens by the hash h so each expert's tokens form (at
most) two contiguous runs (run R_k holds tokens whose experts are k
and k+1). Big runs (>= PAIR_MIN tokens) are processed "paired": both
experts accumulate in one PSUM group and the run writes the combined
partial to zsum. Small runs are processed per-expert over merged
contiguous segments (R_{e-1} u R_e), writing to one of two parity
outputs (each token has exactly one even and one odd expert). The
host stitches zsum / zaux0+zaux1 per column, sums over cores, scales
by 1/2, adds the b2 terms, and un-permutes.

Matmuls run in float32r (full PE rate; values pre-rounded on the host
to the fp32r grid = round-to-nearest-even keeping 11 explicit mantissa
bits). PSUM accumulation is fp32; biases are applied in fp32. Token
chunks are split evenly so each matmul outlasts its ~148 ns LDWEIGHTS
and the PE streams at full rate; the merged segments keep small runs
on large chunk grids too. Weights prefetch ahead of use. Work and
weight traffic are identical on every core: ~17 GFLOP of matmul +
~38 MB weights + ~20 MB tokens in / ~20 MB out.
"""

import os

import numpy as np

import concourse.bass as bass
import concourse.mybir as mybir
import concourse.tile as tile
from concourse import bacc
from concourse.bass_utils import run_bass_kernel_spmd

# Problem shape (nn_MoELayer: HIDDEN=1024, NUM_EXPERTS=8, TOP_K=2, B=2, S=2048)
D = 1024
DFF = 4096
E = 8
N_CORES = 8
FSL = DFF // N_CORES          # dff slice per core = 512
DC = D // 128                 # 8 contraction chunks for mm1
FC = FSL // 128               # 4 dff chunks per slice
DB = D // 128                 # 8 output-row blocks for mm2
MAX_CHUNK = 512               # token chunk (PSUM bank / fp32 moving limit)
PAIR_MIN = 512                # runs >= this are processed expert-paired

f32 = mybir.dt.float32
f32r = mybir.dt.float32r

LAST_RESULTS = None           # set on each kernel() call (exec stats for test.py)


def _round_fp32r(a: np.ndarray) -> np.ndarray:
    """Round fp32 values to the fp32r grid (RNE, keep 11 explicit mantissa
    bits — matches the hardware's fp32->fp32r cast bit-for-bit)."""
    b = np.ascontiguousarray(a, dtype=np.float32).view(np.uint32).astype(np.uint64)
    keep = b & 0xFFFFF000
    rem = b & 0xFFF
    lsb = (b >> 12) & 1
    up = (rem > 0x800) | ((rem == 0x800) & (lsb == 1))
    out = (keep + (up.astype(np.uint64) << 12)) & 0xFFFFFFFF
    return out.astype(np.uint32).view(np.float32).reshape(a.shape)


def _chunks(start: int, length: int) -> list[tuple[int, int]]:
    """Split [start, start+length) (length a multiple of 4) into even-sized
    chunks of <= MAX_CHUNK on a 4 grid. Even splitting keeps chunks large
    so matmul duration exceeds the per-instruction LDWEIGHTS time."""
    if length == 0:
        return []
    n = -(-length // MAX_CHUNK)
    base = (length // n) // 4 * 4
    k = (length - n * base) // 4
    sizes = [base + 4] * k + [base] * (n - k)
    out = []
    pos = start
    for sz in sizes:
        out.append((pos, sz))
        pos += sz
    return out


def _try_install_ntff_hook() -> None:
    """Best-effort install of the axon NTFF profile hook (the container's
    antenv package lacks axon_hooks). Only needed when tracing."""
    import sys
    import types

    try:
        import antenv  # noqa: F401

        if "antenv.axon_hooks" in sys.modules:
            return
        mod = types.ModuleType("antenv.axon_hooks")
        _h = {}
        mod.set_axon_ntff_profile_hook = lambda h: _h.__setitem__("h", h)
        mod.get_axon_ntff_profile_hook = lambda: _h.get("h")
        sys.modules["antenv.axon_hooks"] = mod
        antenv.axon_hooks = mod
        from trn_agent_boot.trn_boot import _ntff_profile_via_ctypes

        mod.set_axon_ntff_profile_hook(
            _ntff_profile_via_ctypes("/opt/axon/libaxon_pjrt.so")
        )
        import concourse.bass_utils as bu

        bu.upload_artifacts = lambda tmpdir: f"local:{tmpdir}"
    except Exception:
        pass


def _plan_tasks(aug_starts, aug_lens):
    """Build the ordered work list. Each task is (experts, c0, cw, out_id):
    out_id 0 = zsum (paired), 1 = zaux0 (even expert), 2 = zaux1 (odd).
    Returns (tasks, paired_mask)."""
    paired = [int(aug_lens[k]) >= PAIR_MIN for k in range(E)]

    items = []   # (sort_key, experts tuple, c0, cw, out_id)
    for k in range(E):
        if aug_lens[k] == 0 or not paired[k]:
            continue
        for (c0, cw) in _chunks(int(aug_starts[k]), int(aug_lens[k])):
            items.append((c0, (k, (k + 1) % E), c0, cw, 0))

    # per-expert segments over unpaired runs: expert e touches run e-1 (as
    # second expert) and run e (as first); adjacent unpaired runs merge
    for e in range(E):
        parts = []
        rprev = (e - 1) % E
        if not paired[rprev] and aug_lens[rprev]:
            parts.append((int(aug_starts[rprev]), int(aug_lens[rprev])))
        if not paired[e] and aug_lens[e]:
            parts.append((int(aug_starts[e]), int(aug_lens[e])))
        if len(parts) == 2 and parts[0][0] + parts[0][1] == parts[1][0]:
            parts = [(parts[0][0], parts[0][1] + parts[1][1])]
        out_id = 1 if e % 2 == 0 else 2
        for (s0, ln) in parts:
            for (c0, cw) in _chunks(s0, ln):
                items.append((c0, (e,), c0, cw, out_id))

    items.sort(key=lambda it: (it[0], it[1]))
    return [(ex, c0, cw, oid) for _, ex, c0, cw, oid in items], paired


def _build_kernel(T: int, tasks):
    """Emit the per-core Bass program. All cores run the same program; only
    the weight-slice input data differs."""
    nc = bacc.Bacc("TRN2", target_bir_lowering=False, debug=False,
                   num_devices=N_CORES)

    xt_ext = nc.dram_tensor("xt", [D, T], f32r, kind="ExternalInput")
    w1_ext = nc.dram_tensor("w1", [E, D, FSL], f32r, kind="ExternalInput")
    b1_ext = nc.dram_tensor("b1", [E, FSL], f32, kind="ExternalInput")
    w2_ext = nc.dram_tensor("w2", [E, FSL, D], f32r, kind="ExternalInput")
    zs_ext = nc.dram_tensor("zsum", [D, T], f32, kind="ExternalOutput")
    za_ext = nc.dram_tensor("zaux0", [D, T], f32, kind="ExternalOutput")
    zb_ext = nc.dram_tensor("zaux1", [D, T], f32, kind="ExternalOutput")

    xt_v = xt_ext.ap().rearrange("(c p) t -> p c t", p=128)   # [128, DC, T]
    z_vs = [
        zs_ext.ap().rearrange("(b p) t -> p b t", p=128),     # [128, DB, T]
        za_ext.ap().rearrange("(b p) t -> p b t", p=128),
        zb_ext.ap().rearrange("(b p) t -> p b t", p=128),
    ]

    relu = mybir.ActivationFunctionType.Relu

    with tile.TileContext(nc) as tc:
        with (
            tc.tile_pool(name="wp", bufs=3) as wp,
            tc.tile_pool(name="bp", bufs=1) as bp,
            tc.tile_pool(name="xp", bufs=3) as xp,
            tc.tile_pool(name="hp", bufs=4) as hp,
            tc.tile_pool(name="zp", bufs=3) as zp,
            tc.tile_pool(name="ps1", bufs=4, space="PSUM") as ps1,
            tc.tile_pool(name="ps2", bufs=4, space="PSUM") as ps2,
        ):
            b1_all = bp.tile([128, E, FC], f32, tag="b1")

            w_tiles: dict[int, tuple] = {}

            def load_expert(e: int):
                w1t = wp.tile([128, DC, FSL], f32r, tag="w1e", name="w1t")
                for kd in range(DC):
                    nc.sync.dma_start(
                        out=w1t[:, kd, :],
                        in_=w1_ext[e, kd * 128:(kd + 1) * 128, :],
                    )
                w2t = wp.tile([128, FC, D], f32r, tag="w2e", name="w2t")
                for fb in range(FC):
                    nc.sync.dma_start(
                        out=w2t[:, fb, :],
                        in_=w2_ext[e, fb * 128:(fb + 1) * 128, :],
                    )
                w_tiles[e] = (w1t, w2t)

            # xc prefetch: issue chunk i's token DMA 2 chunks ahead so the
            # head of the DMA rings carries the critical-path loads
            xcs: dict[int, object] = {}

            def issue_xc(i: int):
                if i >= len(tasks):
                    return
                _, c0, cw, _ = tasks[i]
                xc = xp.tile([128, DC, MAX_CHUNK], f32r, tag="xc", name="xc")
                nc.scalar.dma_start(out=xc[:, :, :cw], in_=xt_v[:, :, c0:c0 + cw])
                xcs[i] = xc

            issue_xc(0)
            nc.scalar.dma_start(
                out=b1_all[:],
                in_=b1_ext.ap().rearrange("e (c p) -> p e c", p=128),
            )
            issue_xc(1)

            for ti, (experts, c0, cw, out_id) in enumerate(tasks):
                # ensure this task's experts are loaded; prefetch the next
                # task's new experts while this one computes
                for e in experts:
                    if e not in w_tiles:
                        load_expert(e)
                upcoming = set(experts)
                for nxt in tasks[ti + 1:ti + 3]:
                    upcoming |= set(nxt[0])
                for stale in [x for x in w_tiles if x not in upcoming]:
                    w_tiles.pop(stale)
                for nxt in tasks[ti + 1:ti + 2]:
                    for e in nxt[0]:
                        if e not in w_tiles and len(w_tiles) < 3:
                            load_expert(e)

                issue_xc(ti + 2)
                xc = xcs.pop(ti)

                # mm1 + relu(+b1): ht[fb, tok] per task expert
                hts = []
                for e in experts:
                    w1t = w_tiles[e][0]
                    ht = hp.tile([128, FC, MAX_CHUNK], f32r, tag="ht", name="ht")
                    for fb in range(FC):
                        acc = ps1.tile([128, MAX_CHUNK], f32, tag="acc1",
                                       name="acc1")
                        for kd in range(DC):
                            nc.tensor.matmul(
                                acc[:, :cw],
                                w1t[:, kd, fb * 128:(fb + 1) * 128],
                                xc[:, kd, :cw],
                                start=(kd == 0),
                                stop=(kd == DC - 1),
                            )
                        nc.scalar.activation(
                            ht[:, fb, :cw],
                            acc[:, :cw],
                            relu,
                            bias=b1_all[:, e, fb:fb + 1],
                        )
                    hts.append(ht)

                # mm2: zT[d, tok] accumulating this task's experts over the
                # dff slice, written to the task's output tensor
                z_v = z_vs[out_id]
                n_acc = len(experts) * FC
                for half in range(2):
                    zs = zp.tile([128, DB // 2, MAX_CHUNK], f32, tag="zs",
                                 name="zs")
                    for dbi in range(DB // 2):
                        db = half * (DB // 2) + dbi
                        acc2 = ps2.tile([128, MAX_CHUNK], f32, tag="acc2",
                                        name="acc2")
                        step = 0
                        for ei, e in enumerate(experts):
                            w2t = w_tiles[e][1]
                            for fb in range(FC):
                                nc.tensor.matmul(
                                    acc2[:, :cw],
                                    w2t[:, fb, db * 128:(db + 1) * 128],
                                    hts[ei][:, fb, :cw],
                                    start=(step == 0),
                                    stop=(step == n_acc - 1),
                                )
                                step += 1
                        nc.vector.tensor_copy(zs[:, dbi, :cw], acc2[:, :cw])
                    nc.gpsimd.dma_start(
                        out=z_v[:, half * (DB // 2):(half + 1) * (DB // 2),
                                c0:c0 + cw],
                        in_=zs[:, :, :cw],
                    )

    nc.compile()
    return nc


def kernel(x: np.ndarray, W1: np.ndarray, b1: np.ndarray,
           W2: np.ndarray, b2: np.ndarray) -> np.ndarray:
    global LAST_RESULTS

    x = np.asarray(x, dtype=np.float32)
    W1 = np.asarray(W1, dtype=np.float32)
    b1 = np.asarray(b1, dtype=np.float32)
    W2 = np.asarray(W2, dtype=np.float32)
    b2 = np.asarray(b2, dtype=np.float32)

    B, S, d = x.shape
    assert d == D and W1.shape == (E, D, DFF) and W2.shape == (E, DFF, D)
    T = B * S
    x_flat = x.reshape(T, D)

    # hash routing. Must match the reference's EAGER jnp ops bit-for-bit:
    # on the neuron/axon backend the eager float->int32 astype rounds to
    # nearest (unlike numpy's truncation), so replicate via the same ops.
    try:
        import jax.numpy as jnp

        h = np.asarray(
            jnp.mod(jnp.asarray(x_flat)[:, :2].sum(axis=1).astype(jnp.int32), E)
        ).astype(np.int64)
    except Exception:
        h = np.mod((x_flat[:, 0] + x_flat[:, 1]).astype(np.int32), E).astype(np.int64)

    # sort tokens by h -> contiguous runs per hash value
    perm = np.argsort(h, kind="stable")
    h_sorted = h[perm]
    run_lens = np.bincount(h_sorted, minlength=E)

    # fp32r matmuls require even/aligned free-dim patterns: pad every run to
    # a multiple of 4 tokens with zero columns (their outputs are discarded)
    pad_lens = (-run_lens) % 4
    aug_lens = run_lens + pad_lens
    aug_starts = np.concatenate([[0], np.cumsum(aug_lens)[:-1]])
    T_aug = int(aug_lens.sum())

    x_sorted_T = x_flat[perm].T                               # [D, T]
    xt = np.zeros((D, T_aug), dtype=np.float32)
    col_orig = np.full(T_aug, -1, dtype=np.int64)             # aug col -> sorted idx
    run_of_col = np.zeros(T_aug, dtype=np.int64)
    pos = 0
    for k in range(E):
        s, l = pos, int(run_lens[k])
        a0, al = int(aug_starts[k]), int(aug_lens[k])
        xt[:, a0:a0 + l] = x_sorted_T[:, s:s + l]
        col_orig[a0:a0 + l] = np.arange(s, s + l)
        run_of_col[a0:a0 + al] = k
        pos += l
    xt = _round_fp32r(xt)

    tasks, paired = _plan_tasks(aug_starts, aug_lens)
    nc = _build_kernel(T_aug, tasks)

    # per-core weight slices along dff
    in_maps = []
    for c in range(N_CORES):
        sl = slice(c * FSL, (c + 1) * FSL)
        in_maps.append({
            "xt": xt,
            "w1": _round_fp32r(np.ascontiguousarray(W1[:, :, sl])),
            "b1": np.ascontiguousarray(b1[:, sl]),
            "w2": _round_fp32r(np.ascontiguousarray(W2[:, sl, :])),
        })

    trace = bool(os.environ.get("MOE_KERNEL_TRACE"))
    if trace:
        _try_install_ntff_hook()
    res = run_bass_kernel_spmd(nc, in_maps, list(range(N_CORES)), trace=trace)
    LAST_RESULTS = res

    # combine: paired-run columns come from zsum; unpaired columns are the
    # sum of the two parity outputs. Sum over cores, drop pads, transpose,
    # halve, add the b2 terms, un-permute.
    paired_col = np.asarray(paired)[run_of_col]               # [T_aug] bool
    z = np.zeros((D, T_aug), dtype=np.float32)
    for c in range(N_CORES):
        r = res.results[c]
        z[:, paired_col] += r["zsum"][:, paired_col]
        z[:, ~paired_col] += r["zaux0"][:, ~paired_col]
        z[:, ~paired_col] += r["zaux1"][:, ~paired_col]
    real = col_orig >= 0
    out_sorted = np.empty((T, D), dtype=np.float32)
    out_sorted[col_orig[real]] = z[:, real].T
    out_sorted *= 0.5
    out_sorted += 0.5 * (b2[h_sorted] + b2[(h_sorted + 1) % E])

    out = np.empty_like(out_sorted)
    out[perm] = out_sorted
    return out.reshape(B, S, D)


# revision 18
# speedup vs baseline: 1.0740x; 1.0740x over previous
"""MoE layer (8 experts, top-2 hash routing) on 8 Trainium2 NeuronCores.

Strategy: shard the FFN along the dff axis (4096 -> 8 slices of 512).
Every core computes, for all routed (token, expert) pairs, the partial
FFN contribution of its dff slice:

    z_core[t] = sum_{e in sel(t)} relu(x[t] @ W1[e][:, S] + b1[e][S]) @ W2[e][S, :]

The host sorts tokens by the hash h so each expert's tokens form (at
most) two contiguous runs (run R_k holds tokens whose experts are k
and k+1). Big runs (>= PAIR_MIN tokens) are processed "paired": both
experts accumulate in one PSUM group and the run writes the combined
partial to zsum. Small runs are processed per-expert over merged
contiguous segments (R_{e-1} u R_e), writing to one of two parity
outputs (each token has exactly one even and one odd expert). The
host stitches zsum / zaux0+zaux1 per column, sums over cores, scales
by 1/2, adds the b2 terms, and un-permutes.

Matmuls run in float32r (full PE rate; values pre-rounded on the host
to the fp32r grid = round-to-nearest-even keeping 11 explicit mantissa
bits). PSUM accumulation is fp32; biases are applied in fp32. Token
chunks are split evenly so each matmul outlasts its ~148 ns LDWEIGHTS
and the PE streams at full rate; the merged segments keep small runs
on large chunk grids too. Weights prefetch ahead of use. Work and
weight traffic are identical on every core: ~17 GFLOP of matmul +
~38 MB weights + ~20 MB tokens in / ~20 MB out.
"""

import os

import numpy as np

import concourse.bass as bass
import concourse.mybir as mybir
import concourse.tile as tile
from concourse import bacc
from concourse.bass_utils import run_bass_kernel_spmd

# Problem shape (nn_MoELayer: HIDDEN=1024, NUM_EXPERTS=8, TOP_K=2, B=2, S=2048)
D = 1024
DFF = 4096
E = 8
N_CORES = 8
FSL = DFF // N_CORES          # dff slice per core = 512
DC = D // 128                 # 8 contraction chunks for mm1
FC = FSL // 128               # 4 dff chunks per slice
DB = D // 128                 # 8 output-row blocks for mm2
MAX_CHUNK = 512               # token chunk (PSUM bank / fp32 moving limit)
PAIR_MIN = 512                # runs >= this are processed expert-paired

f32 = mybir.dt.float32
f32r = mybir.dt.float32r

LAST_RESULTS = None           # set on each kernel() call (exec stats for test.py)


def _round_fp32r(a: np.ndarray) -> np.ndarray:
    """Round fp32 values to the fp32r grid (RNE, keep 11 explicit mantissa
    bits — matches the hardware's fp32->fp32r cast bit-for-bit)."""
    b = np.ascontiguousarray(a, dtype=np.float32).view(np.uint32).astype(np.uint64)
    keep = b & 0xFFFFF000
    rem = b & 0xFFF
    lsb = (b >> 12) & 1
    up = (rem > 0x800) | ((rem == 0x800) & (lsb == 1))
    out = (keep + (up.astype(np.uint64) << 12)) & 0xFFFFFFFF
    return out.astype(np.uint32).view(np.float32).reshape(a.shape)


def _chunks(start: int, length: int) -> list[tuple[int, int]]:
    """Split [start, start+length) (length a multiple of 4) into even-sized
    chunks of <= MAX_CHUNK on a 4 grid. Even splitting keeps chunks large
    so matmul duration exceeds the per-instruction LDWEIGHTS time."""
    if length == 0:
        return []
    n = -(-length // MAX_CHUNK)
    base = (length // n) // 4 * 4
    k = (length - n * base) // 4
    sizes = [base + 4] * k + [base] * (n - k)
    out = []
    pos = start
    for sz in sizes:
        out.append((pos, sz))
        pos += sz
    return out


def _try_install_ntff_hook() -> None:
    """Best-effort install of the axon NTFF profile hook (the container's
    antenv package lacks axon_hooks). Only needed when tracing."""
    import sys
    import types

    try:
        import antenv  # noqa: F401

        if "antenv.axon_hooks" in sys.modules:
            return
        mod = types.ModuleType("antenv.axon_hooks")
        _h = {}
        mod.set_axon_ntff_profile_hook = lambda h: _h.__setitem__("h", h)
        mod.get_axon_ntff_profile_hook = lambda: _h.get("h")
        sys.modules["antenv.axon_hooks"] = mod
        antenv.axon_hooks = mod
        from trn_agent_boot.trn_boot import _ntff_profile_via_ctypes

        mod.set_axon_ntff_profile_hook(
            _ntff_profile_via_ctypes("/opt/axon/libaxon_pjrt.so")
        )
        import concourse.bass_utils as bu

        bu.upload_artifacts = lambda tmpdir: f"local:{tmpdir}"
    except Exception:
        pass


def _plan_tasks(aug_starts, aug_lens):
    """Build the ordered work list. Each task is (experts, c0, cw, out_id):
    out_id 0 = zsum (paired), 1 = zaux0 (even expert), 2 = zaux1 (odd).
    Returns (tasks, paired_mask)."""
    paired = [int(aug_lens[k]) >= PAIR_MIN for k in range(E)]

    items = []   # (sort_key, experts tuple, c0, cw, out_id)
    for k in range(E):
        if aug_lens[k] == 0 or not paired[k]:
            continue
        for (c0, cw) in _chunks(int(aug_starts[k]), int(aug_lens[k])):
            items.append((c0, (k, (k + 1) % E), c0, cw, 0))

    # per-expert segments over unpaired runs: expert e touches run e-1 (as
    # second expert) and run e (as first); adjacent unpaired runs merge
    for e in range(E):
        parts = []
        rprev = (e - 1) % E
        if not paired[rprev] and aug_lens[rprev]:
            parts.append((int(aug_starts[rprev]), int(aug_lens[rprev])))
        if not paired[e] and aug_lens[e]:
            parts.append((int(aug_starts[e]), int(aug_lens[e])))
        if len(parts) == 2 and parts[0][0] + parts[0][1] == parts[1][0]:
            parts = [(parts[0][0], parts[0][1] + parts[1][1])]
        out_id = 1 if e % 2 == 0 else 2
        for (s0, ln) in parts:
            for (c0, cw) in _chunks(s0, ln):
                items.append((c0, (e,), c0, cw, out_id))

    items.sort(key=lambda it: (it[0], it[1]))
    return [(ex, c0, cw, oid) for _, ex, c0, cw, oid in items], paired


def _build_kernel(T: int, tasks):
    """Emit the per-core Bass program. All cores run the same program; only
    the weight-slice input data differs."""
    nc = bacc.Bacc("TRN2", target_bir_lowering=False, debug=False,
                   num_devices=N_CORES)

    xt_ext = nc.dram_tensor("xt", [D, T], f32r, kind="ExternalInput")
    w1_ext = nc.dram_tensor("w1", [E, D, FSL], f32r, kind="ExternalInput")
    b1_ext = nc.dram_tensor("b1", [E, FSL], f32, kind="ExternalInput")
    w2_ext = nc.dram_tensor("w2", [E, FSL, D], f32r, kind="ExternalInput")
    zs_ext = nc.dram_tensor("zsum", [D, T], f32, kind="ExternalOutput")
    za_ext = nc.dram_tensor("zaux0", [D, T], f32, kind="ExternalOutput")
    zb_ext = nc.dram_tensor("zaux1", [D, T], f32, kind="ExternalOutput")

    xt_v = xt_ext.ap().rearrange("(c p) t -> p c t", p=128)   # [128, DC, T]
    z_vs = [
        zs_ext.ap().rearrange("(b p) t -> p b t", p=128),     # [128, DB, T]
        za_ext.ap().rearrange("(b p) t -> p b t", p=128),
        zb_ext.ap().rearrange("(b p) t -> p b t", p=128),
    ]

    relu = mybir.ActivationFunctionType.Relu

    with tile.TileContext(nc) as tc:
        with (
            tc.tile_pool(name="wp", bufs=4) as wp,
            tc.tile_pool(name="bp", bufs=1) as bp,
            tc.tile_pool(name="xp", bufs=2) as xp,
            tc.tile_pool(name="hp", bufs=4) as hp,
            tc.tile_pool(name="zp", bufs=3) as zp,
            tc.tile_pool(name="ps1", bufs=4, space="PSUM") as ps1,
            tc.tile_pool(name="ps2", bufs=4, space="PSUM") as ps2,
        ):
            b1_all = bp.tile([128, E, FC], f32, tag="b1")

            w_tiles: dict[int, tuple] = {}

            def load_expert(e: int):
                w1t = wp.tile([128, DC, FSL], f32r, tag="w1e", name="w1t")
                for kd in range(DC):
                    nc.sync.dma_start(
                        out=w1t[:, kd, :],
                        in_=w1_ext[e, kd * 128:(kd + 1) * 128, :],
                    )
                w2t = wp.tile([128, FC, D], f32r, tag="w2e", name="w2t")
                for fb in range(FC):
                    nc.sync.dma_start(
                        out=w2t[:, fb, :],
                        in_=w2_ext[e, fb * 128:(fb + 1) * 128, :],
                    )
                w_tiles[e] = (w1t, w2t)

            # xc prefetch: issue chunk i's token DMA 2 chunks ahead so the
            # head of the DMA rings carries the critical-path loads
            xcs: dict[int, object] = {}

            def issue_xc(i: int):
                if i >= len(tasks):
                    return
                _, c0, cw, _ = tasks[i]
                xc = xp.tile([128, DC, MAX_CHUNK], f32r, tag="xc", name="xc")
                nc.scalar.dma_start(out=xc[:, :, :cw], in_=xt_v[:, :, c0:c0 + cw])
                xcs[i] = xc

            issue_xc(0)
            nc.scalar.dma_start(
                out=b1_all[:],
                in_=b1_ext.ap().rearrange("e (c p) -> p e c", p=128),
            )
            issue_xc(1)

            for ti, (experts, c0, cw, out_id) in enumerate(tasks):
                # ensure this task's experts are loaded; prefetch the next
                # task's new experts while this one computes
                for e in experts:
                    if e not in w_tiles:
                        load_expert(e)
                upcoming = set(experts)
                for nxt in tasks[ti + 1:ti + 4]:
                    upcoming |= set(nxt[0])
                for stale in [x for x in w_tiles if x not in upcoming]:
                    w_tiles.pop(stale)
                for nxt in tasks[ti + 1:ti + 3]:
                    for e in nxt[0]:
                        if e not in w_tiles and len(w_tiles) < 4:
                            load_expert(e)

                issue_xc(ti + 2)
                xc = xcs.pop(ti)

                # mm1 + relu(+b1): ht[fb, tok] per task expert
                hts = []
                for e in experts:
                    w1t = w_tiles[e][0]
                    ht = hp.tile([128, FC, MAX_CHUNK], f32r, tag="ht", name="ht")
                    for fb in range(FC):
                        acc = ps1.tile([128, MAX_CHUNK], f32, tag="acc1",
                                       name="acc1")
                        for kd in range(DC):
                            nc.tensor.matmul(
                                acc[:, :cw],
                                w1t[:, kd, fb * 128:(fb + 1) * 128],
                                xc[:, kd, :cw],
                                start=(kd == 0),
                                stop=(kd == DC - 1),
                            )
                        nc.scalar.activation(
                            ht[:, fb, :cw],
                            acc[:, :cw],
                            relu,
                            bias=b1_all[:, e, fb:fb + 1],
                        )
                    hts.append(ht)

                # mm2: zT[d, tok] accumulating this task's experts over the
                # dff slice, written to the task's output tensor
                z_v = z_vs[out_id]
                n_acc = len(experts) * FC
                for half in range(4):
                    zs = zp.tile([128, DB // 4, MAX_CHUNK], f32, tag="zs",
                                 name="zs")
                    for dbi in range(DB // 4):
                        db = half * (DB // 4) + dbi
                        acc2 = ps2.tile([128, MAX_CHUNK], f32, tag="acc2",
                                        name="acc2")
                        step = 0
                        for ei, e in enumerate(experts):
                            w2t = w_tiles[e][1]
                            for fb in range(FC):
                                nc.tensor.matmul(
                                    acc2[:, :cw],
                                    w2t[:, fb, db * 128:(db + 1) * 128],
                                    hts[ei][:, fb, :cw],
                                    start=(step == 0),
                                    stop=(step == n_acc - 1),
                                )
                                step += 1
                        nc.vector.tensor_copy(zs[:, dbi, :cw], acc2[:, :cw])
                    nc.gpsimd.dma_start(
                        out=z_v[:, half * (DB // 4):(half + 1) * (DB // 4),
                                c0:c0 + cw],
                        in_=zs[:, :, :cw],
                    )

    nc.compile()
    return nc


def kernel(x: np.ndarray, W1: np.ndarray, b1: np.ndarray,
           W2: np.ndarray, b2: np.ndarray) -> np.ndarray:
    global LAST_RESULTS

    x = np.asarray(x, dtype=np.float32)
    W1 = np.asarray(W1, dtype=np.float32)
    b1 = np.asarray(b1, dtype=np.float32)
    W2 = np.asarray(W2, dtype=np.float32)
    b2 = np.asarray(b2, dtype=np.float32)

    B, S, d = x.shape
    assert d == D and W1.shape == (E, D, DFF) and W2.shape == (E, DFF, D)
    T = B * S
    x_flat = x.reshape(T, D)

    # hash routing. Must match the reference's EAGER jnp ops bit-for-bit:
    # on the neuron/axon backend the eager float->int32 astype rounds to
    # nearest (unlike numpy's truncation), so replicate via the same ops.
    try:
        import jax.numpy as jnp

        h = np.asarray(
            jnp.mod(jnp.asarray(x_flat)[:, :2].sum(axis=1).astype(jnp.int32), E)
        ).astype(np.int64)
    except Exception:
        h = np.mod((x_flat[:, 0] + x_flat[:, 1]).astype(np.int32), E).astype(np.int64)

    # sort tokens by h -> contiguous runs per hash value
    perm = np.argsort(h, kind="stable")
    h_sorted = h[perm]
    run_lens = np.bincount(h_sorted, minlength=E)

    # fp32r matmuls require even/aligned free-dim patterns: pad every run to
    # a multiple of 4 tokens with zero columns (their outputs are discarded)
    pad_lens = (-run_lens) % 4
    aug_lens = run_lens + pad_lens
    aug_starts = np.concatenate([[0], np.cumsum(aug_lens)[:-1]])
    T_aug = int(aug_lens.sum())

    x_sorted_T = x_flat[perm].T                               # [D, T]
    xt = np.zeros((D, T_aug), dtype=np.float32)
    col_orig = np.full(T_aug, -1, dtype=np.int64)             # aug col -> sorted idx
    run_of_col = np.zeros(T_aug, dtype=np.int64)
    pos = 0
    for k in range(E):
        s, l = pos, int(run_lens[k])
        a0, al = int(aug_starts[k]), int(aug_lens[k])
        xt[:, a0:a0 + l] = x_sorted_T[:, s:s + l]
        col_orig[a0:a0 + l] = np.arange(s, s + l)
        run_of_col[a0:a0 + al] = k
        pos += l
    xt = _round_fp32r(xt)

    tasks, paired = _plan_tasks(aug_starts, aug_lens)
    nc = _build_kernel(T_aug, tasks)

    # per-core weight slices along dff
    in_maps = []
    for c in range(N_CORES):
        sl = slice(c * FSL, (c + 1) * FSL)
        in_maps.append({
            "xt": xt,
            "w1": _round_fp32r(np.ascontiguousarray(W1[:, :, sl])),
            "b1": np.ascontiguousarray(b1[:, sl]),
            "w2": _round_fp32r(np.ascontiguousarray(W2[:, sl, :])),
        })

    trace = bool(os.environ.get("MOE_KERNEL_TRACE"))
    if trace:
        _try_install_ntff_hook()
    res = run_bass_kernel_spmd(nc, in_maps, list(range(N_CORES)), trace=trace)
    LAST_RESULTS = res

    # combine: paired-run columns come from zsum; unpaired columns are the
    # sum of the two parity outputs. Sum over cores, drop pads, transpose,
    # halve, add the b2 terms, un-permute.
    paired_col = np.asarray(paired)[run_of_col]               # [T_aug] bool
    z = np.zeros((D, T_aug), dtype=np.float32)
    for c in range(N_CORES):
        r = res.results[c]
        z[:, paired_col] += r["zsum"][:, paired_col]
        z[:, ~paired_col] += r["zaux0"][:, ~paired_col]
        z[:, ~paired_col] += r["zaux1"][:, ~paired_col]
    real = col_orig >= 0
    out_sorted = np.empty((T, D), dtype=np.float32)
    out_sorted[col_orig[real]] = z[:, real].T
    out_sorted *= 0.5
    out_sorted += 0.5 * (b2[h_sorted] + b2[(h_sorted + 1) % E])

    out = np.empty_like(out_sorted)
    out[perm] = out_sorted
    return out.reshape(B, S, D)


# revision 19
# speedup vs baseline: 1.0814x; 1.0069x over previous
"""MoE layer (8 experts, top-2 hash routing) on 8 Trainium2 NeuronCores.

Strategy: shard the FFN along the dff axis (4096 -> 8 slices of 512).
Every core computes, for all routed (token, expert) pairs, the partial
FFN contribution of its dff slice:

    z_core[t] = sum_{e in sel(t)} relu(x[t] @ W1[e][:, S] + b1[e][S]) @ W2[e][S, :]

The host sorts tokens by the hash h so each expert's tokens form (at
most) two contiguous runs (run R_k holds tokens whose experts are k
and k+1). Big runs (>= PAIR_MIN tokens) are processed "paired": both
experts accumulate in one PSUM group and the run writes the combined
partial to zsum. Small runs are processed per-expert over merged
contiguous segments (R_{e-1} u R_e), writing to one of two parity
outputs (each token has exactly one even and one odd expert). The
host stitches zsum / zaux0+zaux1 per column, sums over cores, scales
by 1/2, adds the b2 terms, and un-permutes.

Matmuls run in float32r (full PE rate; values pre-rounded on the host
to the fp32r grid = round-to-nearest-even keeping 11 explicit mantissa
bits). PSUM accumulation is fp32; biases are applied in fp32. Token
chunks are split evenly so each matmul outlasts its ~148 ns LDWEIGHTS
and the PE streams at full rate; the merged segments keep small runs
on large chunk grids too. Weights prefetch ahead of use. Work and
weight traffic are identical on every core: ~17 GFLOP of matmul +
~38 MB weights + ~20 MB tokens in / ~20 MB out.
"""

import os

import numpy as np

import concourse.bass as bass
import concourse.mybir as mybir
import concourse.tile as tile
from concourse import bacc
from concourse.bass_utils import run_bass_kernel_spmd

# Problem shape (nn_MoELayer: HIDDEN=1024, NUM_EXPERTS=8, TOP_K=2, B=2, S=2048)
D = 1024
DFF = 4096
E = 8
N_CORES = 8
FSL = DFF // N_CORES          # dff slice per core = 512
DC = D // 128                 # 8 contraction chunks for mm1
FC = FSL // 128               # 4 dff chunks per slice
DB = D // 128                 # 8 output-row blocks for mm2
MAX_CHUNK = 512               # token chunk (PSUM bank / fp32 moving limit)
PAIR_MIN = 1                # runs >= this are processed expert-paired

f32 = mybir.dt.float32
f32r = mybir.dt.float32r

LAST_RESULTS = None           # set on each kernel() call (exec stats for test.py)


def _round_fp32r(a: np.ndarray) -> np.ndarray:
    """Round fp32 values to the fp32r grid (RNE, keep 11 explicit mantissa
    bits — matches the hardware's fp32->fp32r cast bit-for-bit)."""
    b = np.ascontiguousarray(a, dtype=np.float32).view(np.uint32).astype(np.uint64)
    keep = b & 0xFFFFF000
    rem = b & 0xFFF
    lsb = (b >> 12) & 1
    up = (rem > 0x800) | ((rem == 0x800) & (lsb == 1))
    out = (keep + (up.astype(np.uint64) << 12)) & 0xFFFFFFFF
    return out.astype(np.uint32).view(np.float32).reshape(a.shape)


def _chunks(start: int, length: int) -> list[tuple[int, int]]:
    """Split [start, start+length) (length a multiple of 4) into even-sized
    chunks of <= MAX_CHUNK on a 4 grid. Even splitting keeps chunks large
    so matmul duration exceeds the per-instruction LDWEIGHTS time."""
    if length == 0:
        return []
    n = -(-length // MAX_CHUNK)
    base = (length // n) // 4 * 4
    k = (length - n * base) // 4
    sizes = [base + 4] * k + [base] * (n - k)
    out = []
    pos = start
    for sz in sizes:
        out.append((pos, sz))
        pos += sz
    return out


def _try_install_ntff_hook() -> None:
    """Best-effort install of the axon NTFF profile hook (the container's
    antenv package lacks axon_hooks). Only needed when tracing."""
    import sys
    import types

    try:
        import antenv  # noqa: F401

        if "antenv.axon_hooks" in sys.modules:
            return
        mod = types.ModuleType("antenv.axon_hooks")
        _h = {}
        mod.set_axon_ntff_profile_hook = lambda h: _h.__setitem__("h", h)
        mod.get_axon_ntff_profile_hook = lambda: _h.get("h")
        sys.modules["antenv.axon_hooks"] = mod
        antenv.axon_hooks = mod
        from trn_agent_boot.trn_boot import _ntff_profile_via_ctypes

        mod.set_axon_ntff_profile_hook(
            _ntff_profile_via_ctypes("/opt/axon/libaxon_pjrt.so")
        )
        import concourse.bass_utils as bu

        bu.upload_artifacts = lambda tmpdir: f"local:{tmpdir}"
    except Exception:
        pass


def _plan_tasks(aug_starts, aug_lens):
    """Build the ordered work list. Each task is (experts, c0, cw, out_id):
    out_id 0 = zsum (paired), 1 = zaux0 (even expert), 2 = zaux1 (odd).
    Returns (tasks, paired_mask)."""
    paired = [int(aug_lens[k]) >= PAIR_MIN for k in range(E)]

    items = []   # (sort_key, experts tuple, c0, cw, out_id)
    for k in range(E):
        if aug_lens[k] == 0 or not paired[k]:
            continue
        for (c0, cw) in _chunks(int(aug_starts[k]), int(aug_lens[k])):
            items.append((c0, (k, (k + 1) % E), c0, cw, 0))

    # per-expert segments over unpaired runs: expert e touches run e-1 (as
    # second expert) and run e (as first); adjacent unpaired runs merge
    for e in range(E):
        parts = []
        rprev = (e - 1) % E
        if not paired[rprev] and aug_lens[rprev]:
            parts.append((int(aug_starts[rprev]), int(aug_lens[rprev])))
        if not paired[e] and aug_lens[e]:
            parts.append((int(aug_starts[e]), int(aug_lens[e])))
        if len(parts) == 2 and parts[0][0] + parts[0][1] == parts[1][0]:
            parts = [(parts[0][0], parts[0][1] + parts[1][1])]
        out_id = 1 if e % 2 == 0 else 2
        for (s0, ln) in parts:
            for (c0, cw) in _chunks(s0, ln):
                items.append((c0, (e,), c0, cw, out_id))

    items.sort(key=lambda it: (it[0], it[1]))
    return [(ex, c0, cw, oid) for _, ex, c0, cw, oid in items], paired


def _build_kernel(T: int, tasks):
    """Emit the per-core Bass program. All cores run the same program; only
    the weight-slice input data differs."""
    nc = bacc.Bacc("TRN2", target_bir_lowering=False, debug=False,
                   num_devices=N_CORES)

    xt_ext = nc.dram_tensor("xt", [D, T], f32r, kind="ExternalInput")
    w1_ext = nc.dram_tensor("w1", [E, D, FSL], f32r, kind="ExternalInput")
    b1_ext = nc.dram_tensor("b1", [E, FSL], f32, kind="ExternalInput")
    w2_ext = nc.dram_tensor("w2", [E, FSL, D], f32r, kind="ExternalInput")
    zs_ext = nc.dram_tensor("zsum", [D, T], f32, kind="ExternalOutput")
    za_ext = nc.dram_tensor("zaux0", [D, T], f32, kind="ExternalOutput")
    zb_ext = nc.dram_tensor("zaux1", [D, T], f32, kind="ExternalOutput")

    xt_v = xt_ext.ap().rearrange("(c p) t -> p c t", p=128)   # [128, DC, T]
    z_vs = [
        zs_ext.ap().rearrange("(b p) t -> p b t", p=128),     # [128, DB, T]
        za_ext.ap().rearrange("(b p) t -> p b t", p=128),
        zb_ext.ap().rearrange("(b p) t -> p b t", p=128),
    ]

    relu = mybir.ActivationFunctionType.Relu

    with tile.TileContext(nc) as tc:
        with (
            tc.tile_pool(name="wp", bufs=4) as wp,
            tc.tile_pool(name="bp", bufs=1) as bp,
            tc.tile_pool(name="xp", bufs=2) as xp,
            tc.tile_pool(name="hp", bufs=4) as hp,
            tc.tile_pool(name="zp", bufs=3) as zp,
            tc.tile_pool(name="ps1", bufs=4, space="PSUM") as ps1,
            tc.tile_pool(name="ps2", bufs=4, space="PSUM") as ps2,
        ):
            b1_all = bp.tile([128, E, FC], f32, tag="b1")

            w_tiles: dict[int, tuple] = {}

            def load_expert(e: int):
                w1t = wp.tile([128, DC, FSL], f32r, tag="w1e", name="w1t")
                for kd in range(DC):
                    nc.sync.dma_start(
                        out=w1t[:, kd, :],
                        in_=w1_ext[e, kd * 128:(kd + 1) * 128, :],
                    )
                w2t = wp.tile([128, FC, D], f32r, tag="w2e", name="w2t")
                for fb in range(FC):
                    nc.sync.dma_start(
                        out=w2t[:, fb, :],
                        in_=w2_ext[e, fb * 128:(fb + 1) * 128, :],
                    )
                w_tiles[e] = (w1t, w2t)

            # xc prefetch: issue chunk i's token DMA 2 chunks ahead so the
            # head of the DMA rings carries the critical-path loads
            xcs: dict[int, object] = {}

            def issue_xc(i: int):
                if i >= len(tasks):
                    return
                _, c0, cw, _ = tasks[i]
                xc = xp.tile([128, DC, MAX_CHUNK], f32r, tag="xc", name="xc")
                nc.scalar.dma_start(out=xc[:, :, :cw], in_=xt_v[:, :, c0:c0 + cw])
                xcs[i] = xc

            issue_xc(0)
            nc.scalar.dma_start(
                out=b1_all[:],
                in_=b1_ext.ap().rearrange("e (c p) -> p e c", p=128),
            )
            issue_xc(1)

            for ti, (experts, c0, cw, out_id) in enumerate(tasks):
                # ensure this task's experts are loaded; prefetch the next
                # task's new experts while this one computes
                for e in experts:
                    if e not in w_tiles:
                        load_expert(e)
                upcoming = set(experts)
                for nxt in tasks[ti + 1:ti + 4]:
                    upcoming |= set(nxt[0])
                for stale in [x for x in w_tiles if x not in upcoming]:
                    w_tiles.pop(stale)
                for nxt in tasks[ti + 1:ti + 3]:
                    for e in nxt[0]:
                        if e not in w_tiles and len(w_tiles) < 4:
                            load_expert(e)

                issue_xc(ti + 2)
                xc = xcs.pop(ti)

                # mm1 + relu(+b1): ht[fb, tok] per task expert
                hts = []
                for e in experts:
                    w1t = w_tiles[e][0]
                    ht = hp.tile([128, FC, MAX_CHUNK], f32r, tag="ht", name="ht")
                    for fb in range(FC):
                        acc = ps1.tile([128, MAX_CHUNK], f32, tag="acc1",
                                       name="acc1")
                        for kd in range(DC):
                            nc.tensor.matmul(
                                acc[:, :cw],
                                w1t[:, kd, fb * 128:(fb + 1) * 128],
                                xc[:, kd, :cw],
                                start=(kd == 0),
                                stop=(kd == DC - 1),
                            )
                        nc.scalar.activation(
                            ht[:, fb, :cw],
                            acc[:, :cw],
                            relu,
                            bias=b1_all[:, e, fb:fb + 1],
                        )
                    hts.append(ht)

                # mm2: zT[d, tok] accumulating this task's experts over the
                # dff slice, written to the task's output tensor
                z_v = z_vs[out_id]
                n_acc = len(experts) * FC
                for half in range(4):
                    zs = zp.tile([128, DB // 4, MAX_CHUNK], f32, tag="zs",
                                 name="zs")
                    for dbi in range(DB // 4):
                        db = half * (DB // 4) + dbi
                        acc2 = ps2.tile([128, MAX_CHUNK], f32, tag="acc2",
                                        name="acc2")
                        step = 0
                        for ei, e in enumerate(experts):
                            w2t = w_tiles[e][1]
                            for fb in range(FC):
                                nc.tensor.matmul(
                                    acc2[:, :cw],
                                    w2t[:, fb, db * 128:(db + 1) * 128],
                                    hts[ei][:, fb, :cw],
                                    start=(step == 0),
                                    stop=(step == n_acc - 1),
                                )
                                step += 1
                        nc.vector.tensor_copy(zs[:, dbi, :cw], acc2[:, :cw])
                    nc.gpsimd.dma_start(
                        out=z_v[:, half * (DB // 4):(half + 1) * (DB // 4),
                                c0:c0 + cw],
                        in_=zs[:, :, :cw],
                    )

    nc.compile()
    return nc


def kernel(x: np.ndarray, W1: np.ndarray, b1: np.ndarray,
           W2: np.ndarray, b2: np.ndarray) -> np.ndarray:
    global LAST_RESULTS

    x = np.asarray(x, dtype=np.float32)
    W1 = np.asarray(W1, dtype=np.float32)
    b1 = np.asarray(b1, dtype=np.float32)
    W2 = np.asarray(W2, dtype=np.float32)
    b2 = np.asarray(b2, dtype=np.float32)

    B, S, d = x.shape
    assert d == D and W1.shape == (E, D, DFF) and W2.shape == (E, DFF, D)
    T = B * S
    x_flat = x.reshape(T, D)

    # hash routing. Must match the reference's EAGER jnp ops bit-for-bit:
    # on the neuron/axon backend the eager float->int32 astype rounds to
    # nearest (unlike numpy's truncation), so replicate via the same ops.
    try:
        import jax.numpy as jnp

        h = np.asarray(
            jnp.mod(jnp.asarray(x_flat)[:, :2].sum(axis=1).astype(jnp.int32), E)
        ).astype(np.int64)
    except Exception:
        h = np.mod((x_flat[:, 0] + x_flat[:, 1]).astype(np.int32), E).astype(np.int64)

    # sort tokens by h -> contiguous runs per hash value
    perm = np.argsort(h, kind="stable")
    h_sorted = h[perm]
    run_lens = np.bincount(h_sorted, minlength=E)

    # fp32r matmuls require even/aligned free-dim patterns: pad every run to
    # a multiple of 4 tokens with zero columns (their outputs are discarded)
    pad_lens = (-run_lens) % 4
    aug_lens = run_lens + pad_lens
    aug_starts = np.concatenate([[0], np.cumsum(aug_lens)[:-1]])
    T_aug = int(aug_lens.sum())

    x_sorted_T = x_flat[perm].T                               # [D, T]
    xt = np.zeros((D, T_aug), dtype=np.float32)
    col_orig = np.full(T_aug, -1, dtype=np.int64)             # aug col -> sorted idx
    run_of_col = np.zeros(T_aug, dtype=np.int64)
    pos = 0
    for k in range(E):
        s, l = pos, int(run_lens[k])
        a0, al = int(aug_starts[k]), int(aug_lens[k])
        xt[:, a0:a0 + l] = x_sorted_T[:, s:s + l]
        col_orig[a0:a0 + l] = np.arange(s, s + l)
        run_of_col[a0:a0 + al] = k
        pos += l
    xt = _round_fp32r(xt)

    tasks, paired = _plan_tasks(aug_starts, aug_lens)
    nc = _build_kernel(T_aug, tasks)

    # per-core weight slices along dff
    in_maps = []
    for c in range(N_CORES):
        sl = slice(c * FSL, (c + 1) * FSL)
        in_maps.append({
            "xt": xt,
            "w1": _round_fp32r(np.ascontiguousarray(W1[:, :, sl])),
            "b1": np.ascontiguousarray(b1[:, sl]),
            "w2": _round_fp32r(np.ascontiguousarray(W2[:, sl, :])),
        })

    trace = bool(os.environ.get("MOE_KERNEL_TRACE"))
    if trace:
        _try_install_ntff_hook()
    res = run_bass_kernel_spmd(nc, in_maps, list(range(N_CORES)), trace=trace)
    LAST_RESULTS = res

    # combine: paired-run columns come from zsum; unpaired columns are the
    # sum of the two parity outputs. Sum over cores, drop pads, transpose,
    # halve, add the b2 terms, un-permute.
    paired_col = np.asarray(paired)[run_of_col]               # [T_aug] bool
    z = np.zeros((D, T_aug), dtype=np.float32)
    for c in range(N_CORES):
        r = res.results[c]
        z[:, paired_col] += r["zsum"][:, paired_col]
        z[:, ~paired_col] += r["zaux0"][:, ~paired_col]
        z[:, ~paired_col] += r["zaux1"][:, ~paired_col]
    real = col_orig >= 0
    out_sorted = np.empty((T, D), dtype=np.float32)
    out_sorted[col_orig[real]] = z[:, real].T
    out_sorted *= 0.5
    out_sorted += 0.5 * (b2[h_sorted] + b2[(h_sorted + 1) % E])

    out = np.empty_like(out_sorted)
    out[perm] = out_sorted
    return out.reshape(B, S, D)
